# revision 5
# baseline (speedup 1.0000x reference)
"""DynamicConv2d (moe_routing) Trainium2 Bass kernel.

Full-input contract: kernel(**inputs) -> np.ndarray [1, 512, 56, 56].

Sharding: 64 conv output channels per core across 8 cores; outputs gathered
on host along the channel dim. No cross-core collectives.

Design (v2):
  - x staged as 14 per-chunk SBUF tiles (7 spatial chunks x 2 channel halves,
    10 rows each with 2-row overlap) so every conv chunk depends on exactly
    one DMA; x halves split across the sync and gpsimd rings, hash tables on
    the vector ring.
  - Only 2 dummy warm-up matmuls; the PE clock ramp completes on real conv
    matmuls (conv starts as soon as chunk-0 data lands).
  - Routing chain fully inside the conv window: projw/sigw/projq/sigq/histc
    matmuls squeezed between conv chunks; qsum built incrementally from the
    per-chunk x tiles (DVE kc0 / gpsimd kc1); rqt in fp16 (sign margin 160x).
  - No PSUM->SBUF staging: bn_stats and the final affine read PSUM directly;
    chunk 6 accumulates as two 224-col halves so its stats half-overlaps the
    last matmuls.
  - Tail: bn_aggr -> sqrt -> recip -> scale/shift, then relu(scale*y+shift)
    split across Scalar-ACT and DVE, out-DMA rotating sync/gpsimd/tensor.
"""

import numpy as np
from contextlib import ExitStack

import concourse.bass as bass
import concourse.mybir as mybir
import concourse.tile as tile
from concourse import bacc
from concourse.bass_utils import run_bass_kernel_spmd

F32 = mybir.dt.float32
F16 = mybir.dt.float16
ALU = mybir.AluOpType
ACT = mybir.ActivationFunctionType

N_CORES = 8
O, C, KK, H, W = 512, 256, 3, 56, 56
OC = O // N_CORES          # 64 out channels per core
S = H * W                  # 3136
HP = H + 2                 # 58 padded
T, HASH = 10, 8
TH = T * HASH              # 80
KD = 18                    # hash contraction chunks (2304/128)
NCH = 7                    # spatial chunks
CH = S // NCH              # 448 columns per PSUM chunk (8 rows of 56)
EPS = 1e-3

_CACHE = {}


def _emit(nc):
    xin = nc.dram_tensor("xin", [C, HP, HP], F16, kind="ExternalInput").ap()
    wconv = nc.dram_tensor("wconv", [128, 2, 9, OC], F16, kind="ExternalInput").ap()
    rmt = nc.dram_tensor("rmt", [128, KD, TH], F16, kind="ExternalInput").ap()
    rqt = nc.dram_tensor("rqt", [128, 2, TH], F16, kind="ExternalInput").ap()
    sigw = nc.dram_tensor("sigw", [TH, T], F16, kind="ExternalInput").ap()
    gamma = nc.dram_tensor("gamma", [OC, 1], F32, kind="ExternalInput").ap()
    beta = nc.dram_tensor("beta", [OC, 1], F32, kind="ExternalInput").ap()
    yout = nc.dram_tensor("yout", [OC, S], F16, kind="ExternalOutput").ap()

    with tile.TileContext(nc) as tc, ExitStack() as ctx:
        consts = ctx.enter_context(tc.tile_pool(name="consts", bufs=1))
        work = ctx.enter_context(tc.tile_pool(name="work", bufs=1))
        pconv = ctx.enter_context(tc.tile_pool(name="pconv", bufs=7, space="PSUM"))
        psm = ctx.enter_context(tc.tile_pool(name="psm", bufs=1, space="PSUM"))

        # ---- warm-up operands: first thing on the DVE queue ----
        wu_l_sb = consts.tile([128, OC], F16)
        nc.vector.memset(wu_l_sb, 0.0)
        wu_r_sb = consts.tile([128, CH], F16)
        nc.vector.memset(wu_r_sb, 0.0)
        eps_sb = consts.tile([OC, 1], F32)
        nc.vector.memset(eps_sb, EPS)
        ones10_sb = consts.tile([T, 1], F16)
        nc.vector.memset(ones10_sb, 1.0)

        # ---- hash tables + bn params on the vector ring ----
        rmt_sb = consts.tile([128, KD, TH], F16)
        nc.scalar.dma_start(out=rmt_sb, in_=rmt)
        rqt_sb = consts.tile([128, 2, TH], F16)
        nc.scalar.dma_start(out=rqt_sb, in_=rqt)
        sigw_sb = consts.tile([TH, T], F16)
        nc.scalar.dma_start(out=sigw_sb, in_=sigw)
        gamma_sb = consts.tile([OC, 1], F32)
        nc.scalar.dma_start(out=gamma_sb, in_=gamma)
        beta_sb = consts.tile([OC, 1], F32)
        nc.scalar.dma_start(out=beta_sb, in_=beta)

        # ---- conv weights + per-chunk x tiles on sync (kc0) / gpsimd (kc1) ----
        wconv_sb = consts.tile([128, 2, 9, OC], F16)
        nc.sync.dma_start(out=wconv_sb[:, 0], in_=wconv[:, 0])
        nc.gpsimd.dma_start(out=wconv_sb[:, 1], in_=wconv[:, 1])
        ring = [nc.sync, nc.gpsimd]
        xc = [[None] * NCH, [None] * NCH]
        for n in range(NCH):
            for kc in range(2):
                t_ = consts.tile([128, 10, HP], F16, tag=f"xc{kc}_{n}", name=f"xc{kc}_{n}")
                xc[kc][n] = t_
                ring[kc].dma_start(
                    out=t_, in_=xin[kc * 128 : (kc + 1) * 128, 8 * n : 8 * n + 10]
                )

        # ---- PE warm-up: 2 dummy matmuls, then real work finishes the ramp ----
        wu_ps = psm.tile([OC, CH], F32, tag="sp", name="wu")
        for i in range(2):
            nc.tensor.matmul(
                wu_ps, lhsT=wu_l_sb, rhs=wu_r_sb, start=(i == 0), stop=(i == 1)
            )

        yraw_sb = work.tile([OC, S], F16)
        stats_sb = work.tile([OC, 8, 6], F32)

        # ---- qsum partials: kc1 on Scalar (activation accum), kc0 on DVE ----
        qpart_sb = work.tile([128, 2, NCH], F32)
        qsumf_sb = work.tile([128, 2], F32)
        qsum_sb = work.tile([128, 2], F16)
        qscr_sb = work.tile([128, 10, HP], F16)

        def qsum_partial(kc, n):
            rows = 8 if n < NCH - 1 else 10
            if kc == 1:
                nc.scalar.activation(
                    qscr_sb[:, 0:rows, :],
                    xc[kc][n][:, 0:rows, :],
                    ACT.Copy,
                    accum_out=qpart_sb[:, kc, n : n + 1],
                )
            else:
                nc.vector.tensor_reduce(
                    out=qpart_sb[:, kc, n : n + 1],
                    in_=xc[kc][n][:, 0:rows, :],
                    axis=mybir.AxisListType.XY,
                    op=ALU.add,
                )

        def qsum_final(kc):
            nc.vector.tensor_reduce(
                out=qsumf_sb[:, kc : kc + 1],
                in_=qpart_sb[:, kc],
                axis=mybir.AxisListType.X,
                op=ALU.add,
            )

        for n in range(NCH):
            qsum_partial(1, n)

        accs = {}

        def conv_chunk(n, splits=1):
            acc = pconv.tile([OC, CH], F32, tag="acc", name=f"acc{n}")
            accs[n] = acc
            cw = CH // splits
            rw = 8 // splits
            for s_ in range(splits):
                for kc in range(2):
                    for t in range(9):
                        ky, kx = t // 3, t % 3
                        nc.tensor.matmul(
                            acc[:, s_ * cw : (s_ + 1) * cw],
                            lhsT=wconv_sb[:, kc, t, :],
                            rhs=xc[kc][n][:, ky + rw * s_ : ky + rw * (s_ + 1), kx : kx + W],
                            start=(kc == 0 and t == 0),
                            stop=(kc == 1 and t == 8),
                        )
                nc.vector.bn_stats(
                    out=stats_sb[:, n + s_, :], in_=acc[:, s_ * cw : (s_ + 1) * cw]
                )

        # ---- conv chunks with routing matmuls squeezed between ----
        for n in range(NCH):
            qsum_partial(0, n)
        conv_chunk(0)
        conv_chunk(1)

        projw_ps = psm.tile([TH, OC], F32, tag="sp", name="projw")
        for kc in range(2):
            for t in range(9):
                nc.tensor.matmul(
                    projw_ps,
                    lhsT=rmt_sb[:, kc * 9 + t, :],
                    rhs=wconv_sb[:, kc, t, :],
                    start=(kc == 0 and t == 0),
                    stop=(kc == 1 and t == 8),
                )
        bits_w = work.tile([TH, OC], F16)
        nc.vector.tensor_scalar(bits_w, projw_ps, 0.0, None, ALU.is_gt)
        qsum_final(0)
        qsum_final(1)
        nc.vector.tensor_copy(qsum_sb, qsumf_sb)

        conv_chunk(2)

        sigw_ps = psm.tile([T, OC], F32, tag="sp", name="sigw")
        nc.tensor.matmul(sigw_ps, lhsT=sigw_sb, rhs=bits_w, start=True, stop=True)
        sigw_cp = work.tile([T, OC], F32)
        nc.scalar.copy(sigw_cp, sigw_ps)

        conv_chunk(3)

        projq_ps = psm.tile([TH, 1], F32, tag="sp", name="projq")
        for kc in range(2):
            nc.tensor.matmul(
                projq_ps,
                lhsT=rqt_sb[:, kc, :],
                rhs=qsum_sb[:, kc : kc + 1],
                start=(kc == 0),
                stop=(kc == 1),
            )
        bits_q = work.tile([TH, 1], F16)
        nc.vector.tensor_scalar(bits_q, projq_ps, 0.0, None, ALU.is_gt)

        conv_chunk(4)

        sigq_ps = psm.tile([T, 1], F32, tag="sp", name="sigq")
        nc.tensor.matmul(sigq_ps, lhsT=sigw_sb, rhs=bits_q, start=True, stop=True)
        sigq_sb = work.tile([T, 1], F32)
        nc.scalar.copy(sigq_sb, sigq_ps)
        match_sb = work.tile([T, OC], F16)
        nc.vector.tensor_scalar(match_sb, sigw_cp, sigq_sb, None, ALU.is_equal)

        conv_chunk(5)

        # hist > 0 mask (top-256 cap can't bind below 256 positives)
        histc_ps = psm.tile([OC, 1], F32, tag="sp", name="histc")
        nc.tensor.matmul(histc_ps, lhsT=match_sb, rhs=ones10_sb, start=True, stop=True)
        mask_sb = work.tile([OC, 1], F32)
        nc.vector.tensor_scalar(mask_sb, histc_ps, 0.0, None, ALU.is_gt)

        conv_chunk(6, splits=2)

        # ---- BN scale/shift ----
        mv_sb = work.tile([OC, 2], F32)
        nc.vector.bn_aggr(out=mv_sb, in_=stats_sb.rearrange("p a b -> p (a b)"))
        std_sb = work.tile([OC, 1], F32)
        nc.scalar.activation(std_sb, mv_sb[:, 1:2], ACT.Sqrt, bias=eps_sb)
        rstd_sb = work.tile([OC, 1], F32)
        nc.vector.reciprocal(rstd_sb, std_sb)
        scale_sb = work.tile([OC, 1], F32)
        nc.vector.scalar_tensor_tensor(
            out=scale_sb,
            in0=gamma_sb,
            scalar=rstd_sb,
            in1=mask_sb,
            op0=ALU.mult,
            op1=ALU.mult,
        )
        msc_sb = work.tile([OC, 1], F32)
        nc.vector.tensor_tensor(msc_sb, mv_sb[:, 0:1], scale_sb, ALU.mult)
        shift_sb = work.tile([OC, 1], F32)
        nc.vector.tensor_tensor(shift_sb, beta_sb, msc_sb, ALU.subtract)

        # ---- final relu(scale*y+shift) straight from PSUM, ACT + DVE lanes ----
        out_engs = [nc.sync, nc.gpsimd, nc.scalar]

        def affine_act(n, sl0=0, sl1=CH):
            gl = slice(n * CH + sl0, n * CH + sl1)
            nc.scalar.activation(
                yraw_sb[:, gl], accs[n][:, sl0:sl1], ACT.Relu,
                bias=shift_sb, scale=scale_sb,
            )
            return gl

        def affine_dve(n, sl0=0, sl1=CH):
            gl = slice(n * CH + sl0, n * CH + sl1)
            nc.vector.tensor_scalar(
                yraw_sb[:, gl], accs[n][:, sl0:sl1], scale_sb, shift_sb, ALU.mult,
                op1=ALU.add,
            )
            nc.vector.tensor_scalar_max(yraw_sb[:, gl], yraw_sb[:, gl], 0.0)
            return gl

        plan = [
            (affine_act, 0, 0, CH), (affine_dve, 1, 0, CH),
            (affine_act, 2, 0, CH), (affine_dve, 3, 0, CH),
            (affine_act, 4, 0, CH), (affine_dve, 5, 0, CH),
            (affine_act, 6, 0, CH // 2), (affine_dve, 6, CH // 2, CH),
        ]
        for i, (fn, n, a, b) in enumerate(plan):
            gl = fn(n, a, b)
            out_engs[i % 3].dma_start(out=yout[:, gl], in_=yraw_sb[:, gl])

    return nc


def build_nc():
    if "nc" not in _CACHE:
        nc = bacc.Bacc("TRN2", target_bir_lowering=False, debug=False)
        _emit(nc)
        nc.compile()
        _CACHE["nc"] = nc
    return _CACHE["nc"]


def make_in_maps(x, whole_w, rm_w, rm_q, bn_gamma, bn_beta):
    x = np.asarray(x, np.float32)
    whole_w = np.asarray(whole_w, np.float32)
    rm_w = np.asarray(rm_w, np.float32)
    rm_q = np.asarray(rm_q, np.float32)
    bn_gamma = np.asarray(bn_gamma, np.float32)
    bn_beta = np.asarray(bn_beta, np.float32)

    x0 = np.zeros((C, HP, HP), np.float32)
    x0[:, 1 : HP - 1, 1 : HP - 1] = x[0]
    x0 = x0.astype(np.float16)
    wc9 = whole_w.reshape(O, C, 9)
    rmt_a = np.ascontiguousarray(
        rm_w.reshape(TH, 2, 128, 9).transpose(2, 1, 3, 0).reshape(128, KD, TH)
    ).astype(np.float16)
    rqt_a = np.ascontiguousarray(
        rm_q.reshape(TH, C).T.reshape(2, 128, TH).transpose(1, 0, 2)
    ).astype(np.float16)
    sigw_a = np.zeros((TH, T), np.float32)
    for t in range(T):
        for h in range(HASH):
            sigw_a[t * HASH + h, t] = float(2 ** (HASH - 1 - h))
    sigw_a = sigw_a.astype(np.float16)

    in_maps = []
    for core in range(N_CORES):
        o0 = core * OC
        wconv_a = np.ascontiguousarray(
            wc9[o0 : o0 + OC].reshape(OC, 2, 128, 9).transpose(2, 1, 3, 0)
        ).astype(np.float16)
        in_maps.append(
            {
                "xin": x0,
                "wconv": wconv_a,
                "rmt": rmt_a,
                "rqt": rqt_a,
                "sigw": sigw_a,
                "gamma": np.ascontiguousarray(bn_gamma[o0 : o0 + OC, None]),
                "beta": np.ascontiguousarray(bn_beta[o0 : o0 + OC, None]),
            }
        )
    return in_maps


def kernel(x, whole_w, rm_w, rm_q, bn_gamma, bn_beta):
    nc = build_nc()
    in_maps = make_in_maps(x, whole_w, rm_w, rm_q, bn_gamma, bn_beta)
    res = run_bass_kernel_spmd(nc, in_maps, list(range(N_CORES)))
    y = np.concatenate([r["yout"] for r in res.results], axis=0)
    return y.reshape(1, O, H, W).astype(np.float32)


# revision 6
# speedup vs baseline: 1.0735x; 1.0735x over previous
"""DynamicConv2d (moe_routing) Trainium2 Bass kernel.

Full-input contract: kernel(**inputs) -> np.ndarray [1, 512, 56, 56].

Sharding: 64 conv output channels per core across 8 cores; outputs gathered
on host along the channel dim. No cross-core collectives.

Design (v2):
  - x staged as 14 per-chunk SBUF tiles (7 spatial chunks x 2 channel halves,
    10 rows each with 2-row overlap) so every conv chunk depends on exactly
    one DMA; x halves split across the sync and gpsimd rings, hash tables on
    the vector ring.
  - Only 2 dummy warm-up matmuls; the PE clock ramp completes on real conv
    matmuls (conv starts as soon as chunk-0 data lands).
  - Routing chain fully inside the conv window: projw/sigw/projq/sigq/histc
    matmuls squeezed between conv chunks; qsum built incrementally from the
    per-chunk x tiles (DVE kc0 / gpsimd kc1); rqt in fp16 (sign margin 160x).
  - No PSUM->SBUF staging: bn_stats and the final affine read PSUM directly;
    chunk 6 accumulates as two 224-col halves so its stats half-overlaps the
    last matmuls.
  - Tail: bn_aggr -> sqrt -> recip -> scale/shift, then relu(scale*y+shift)
    split across Scalar-ACT and DVE, out-DMA rotating sync/gpsimd/tensor.
"""

import numpy as np
from contextlib import ExitStack

import concourse.bass as bass
import concourse.mybir as mybir
import concourse.tile as tile
from concourse import bacc
from concourse.bass_utils import run_bass_kernel_spmd

F32 = mybir.dt.float32
F16 = mybir.dt.float16
ALU = mybir.AluOpType
ACT = mybir.ActivationFunctionType

N_CORES = 8
O, C, KK, H, W = 512, 256, 3, 56, 56
OC = O // N_CORES          # 64 out channels per core
S = H * W                  # 3136
HP = H + 2                 # 58 padded
T, HASH = 10, 8
TH = T * HASH              # 80
KD = 18                    # hash contraction chunks (2304/128)
NCH = 7                    # spatial chunks
CH = S // NCH              # 448 columns per PSUM chunk (8 rows of 56)
EPS = 1e-3

_CACHE = {}


def _emit(nc):
    xin = nc.dram_tensor("xin", [C, HP, HP], F16, kind="ExternalInput").ap()
    wconv = nc.dram_tensor("wconv", [128, 2, 9, OC], F16, kind="ExternalInput").ap()
    blob16 = nc.dram_tensor("blob16", [128, KD * TH + 2 * TH + T], F16, kind="ExternalInput").ap()
    blob32 = nc.dram_tensor("blob32", [128, 2], F32, kind="ExternalInput").ap()
    yout = nc.dram_tensor("yout", [OC, S], F16, kind="ExternalOutput").ap()

    with tile.TileContext(nc) as tc, ExitStack() as ctx:
        consts = ctx.enter_context(tc.tile_pool(name="consts", bufs=1))
        work = ctx.enter_context(tc.tile_pool(name="work", bufs=1))
        pconv = ctx.enter_context(tc.tile_pool(name="pconv", bufs=7, space="PSUM"))
        psm = ctx.enter_context(tc.tile_pool(name="psm", bufs=1, space="PSUM"))

        # ---- warm-up operands: first thing on the DVE queue ----
        wu_l_sb = consts.tile([128, OC], F16)
        nc.vector.memset(wu_l_sb, 0.0)
        wu_r_sb = consts.tile([128, CH], F16)
        nc.vector.memset(wu_r_sb, 0.0)
        eps_sb = consts.tile([OC, 1], F32)
        nc.vector.memset(eps_sb, EPS)
        ones10_sb = consts.tile([T, 1], F16)
        nc.vector.memset(ones10_sb, 1.0)

        # ---- hash tables + bn params: two packed DMAs on the scalar ring ----
        blob16_sb = consts.tile([128, KD * TH + 2 * TH + T], F16)
        nc.scalar.dma_start(out=blob16_sb, in_=blob16)
        blob32_sb = consts.tile([128, 2], F32)
        nc.scalar.dma_start(out=blob32_sb, in_=blob32)
        rmt_sb = blob16_sb  # [:, (kc*9+t)*TH : +TH]
        rqt0 = KD * TH
        sigw_sb = blob16_sb[0:TH, KD * TH + 2 * TH : KD * TH + 2 * TH + T]
        gamma_sb = blob32_sb[0:OC, 0:1]
        beta_sb = blob32_sb[0:OC, 1:2]

        # ---- conv weights + per-chunk x tiles on sync (kc0) / gpsimd (kc1) ----
        wconv_sb = consts.tile([128, 2, 9, OC], F16)
        nc.sync.dma_start(out=wconv_sb[:, 0], in_=wconv[:, 0])
        nc.gpsimd.dma_start(out=wconv_sb[:, 1], in_=wconv[:, 1])
        ring = [nc.sync, nc.gpsimd]
        xc = [[None] * NCH, [None] * NCH]
        for n in range(NCH):
            for kc in range(2):
                t_ = consts.tile([128, 10, HP], F16, tag=f"xc{kc}_{n}", name=f"xc{kc}_{n}")
                xc[kc][n] = t_
                ring[kc].dma_start(
                    out=t_, in_=xin[kc * 128 : (kc + 1) * 128, 8 * n : 8 * n + 10]
                )

        # ---- PE warm-up: ramp the p-state with continuous dummy matmuls ----
        wu_ps = psm.tile([OC, CH], F32, tag="sp", name="wu")
        NWU = 8
        for i in range(NWU):
            nc.tensor.matmul(
                wu_ps, lhsT=wu_l_sb, rhs=wu_r_sb, start=(i == 0), stop=(i == NWU - 1)
            )

        yraw_sb = work.tile([OC, S], F16)
        stats_sb = work.tile([OC, 8, 6], F32)

        # ---- qsum partials: kc1 on Scalar (activation accum), kc0 on DVE ----
        qpart_sb = work.tile([128, 2, NCH], F32)
        qsumf_sb = work.tile([128, 2], F32)
        qsum_sb = work.tile([128, 2], F16)
        qscr_sb = work.tile([128, 10, HP], F16)

        def qsum_partial(kc, n):
            rows = 8 if n < NCH - 1 else 10
            if kc == 1:
                nc.scalar.activation(
                    qscr_sb[:, 0:rows, :],
                    xc[kc][n][:, 0:rows, :],
                    ACT.Copy,
                    accum_out=qpart_sb[:, kc, n : n + 1],
                )
            else:
                nc.vector.tensor_reduce(
                    out=qpart_sb[:, kc, n : n + 1],
                    in_=xc[kc][n][:, 0:rows, :],
                    axis=mybir.AxisListType.XY,
                    op=ALU.add,
                )

        def qsum_final(kc):
            nc.vector.tensor_reduce(
                out=qsumf_sb[:, kc : kc + 1],
                in_=qpart_sb[:, kc],
                axis=mybir.AxisListType.X,
                op=ALU.add,
            )

        for n in range(NCH):
            qsum_partial(1, n)

        accs = {}

        def conv_chunk(n, splits=1):
            acc = pconv.tile([OC, CH], F32, tag="acc", name=f"acc{n}")
            accs[n] = acc
            cw = CH // splits
            rw = 8 // splits
            for s_ in range(splits):
                for kc in range(2):
                    for t in range(9):
                        ky, kx = t // 3, t % 3
                        nc.tensor.matmul(
                            acc[:, s_ * cw : (s_ + 1) * cw],
                            lhsT=wconv_sb[:, kc, t, :],
                            rhs=xc[kc][n][:, ky + rw * s_ : ky + rw * (s_ + 1), kx : kx + W],
                            start=(kc == 0 and t == 0),
                            stop=(kc == 1 and t == 8),
                        )
                nc.vector.bn_stats(
                    out=stats_sb[:, n + s_, :], in_=acc[:, s_ * cw : (s_ + 1) * cw]
                )

        # ---- conv chunks with routing matmuls squeezed between ----
        for n in range(NCH):
            qsum_partial(0, n)
        conv_chunk(0)
        conv_chunk(1)

        projw_ps = psm.tile([TH, OC], F32, tag="sp", name="projw")
        for kc in range(2):
            for t in range(9):
                nc.tensor.matmul(
                    projw_ps,
                    lhsT=rmt_sb[:, (kc * 9 + t) * TH : (kc * 9 + t + 1) * TH],
                    rhs=wconv_sb[:, kc, t, :],
                    start=(kc == 0 and t == 0),
                    stop=(kc == 1 and t == 8),
                )
        bits_w = work.tile([TH, OC], F16)
        nc.vector.tensor_scalar(bits_w, projw_ps, 0.0, None, ALU.is_gt)
        qsum_final(0)
        qsum_final(1)
        nc.vector.tensor_copy(qsum_sb, qsumf_sb)

        conv_chunk(2)

        sigw_ps = psm.tile([T, OC], F32, tag="sp", name="sigw")
        nc.tensor.matmul(sigw_ps, lhsT=sigw_sb, rhs=bits_w, start=True, stop=True)
        sigw_cp = work.tile([T, OC], F32)
        nc.scalar.copy(sigw_cp, sigw_ps)

        conv_chunk(3)

        projq_ps = psm.tile([TH, 1], F32, tag="sp", name="projq")
        for kc in range(2):
            nc.tensor.matmul(
                projq_ps,
                lhsT=blob16_sb[:, rqt0 + kc * TH : rqt0 + (kc + 1) * TH],
                rhs=qsum_sb[:, kc : kc + 1],
                start=(kc == 0),
                stop=(kc == 1),
            )
        bits_q = work.tile([TH, 1], F16)
        nc.vector.tensor_scalar(bits_q, projq_ps, 0.0, None, ALU.is_gt)

        conv_chunk(4)

        sigq_ps = psm.tile([T, 1], F32, tag="sp", name="sigq")
        nc.tensor.matmul(sigq_ps, lhsT=sigw_sb, rhs=bits_q, start=True, stop=True)
        sigq_sb = work.tile([T, 1], F32)
        nc.scalar.copy(sigq_sb, sigq_ps)

        conv_chunk(5)

        match_sb = work.tile([T, OC], F16)
        nc.vector.tensor_scalar(match_sb, sigw_cp, sigq_sb, None, ALU.is_equal)
        # hist > 0 mask (top-256 cap can't bind below 256 positives)
        histc_ps = psm.tile([OC, 1], F32, tag="sp", name="histc")
        nc.tensor.matmul(histc_ps, lhsT=match_sb, rhs=ones10_sb, start=True, stop=True)
        mask_sb = work.tile([OC, 1], F32)
        nc.vector.tensor_scalar(mask_sb, histc_ps, 0.0, None, ALU.is_gt)

        conv_chunk(6, splits=2)

        # ---- BN scale/shift ----
        mv_sb = work.tile([OC, 2], F32)
        nc.vector.bn_aggr(out=mv_sb, in_=stats_sb.rearrange("p a b -> p (a b)"))
        std_sb = work.tile([OC, 1], F32)
        nc.scalar.activation(std_sb, mv_sb[:, 1:2], ACT.Sqrt, bias=eps_sb)
        rstd_sb = work.tile([OC, 1], F32)
        nc.vector.reciprocal(rstd_sb, std_sb)
        scale_sb = work.tile([OC, 1], F32)
        nc.vector.scalar_tensor_tensor(
            out=scale_sb,
            in0=gamma_sb,
            scalar=rstd_sb,
            in1=mask_sb,
            op0=ALU.mult,
            op1=ALU.mult,
        )
        msc_sb = work.tile([OC, 1], F32)
        nc.vector.tensor_tensor(msc_sb, mv_sb[:, 0:1], scale_sb, ALU.mult)
        shift_sb = work.tile([OC, 1], F32)
        nc.vector.tensor_tensor(shift_sb, beta_sb, msc_sb, ALU.subtract)

        # ---- final relu(scale*y+shift) straight from PSUM, ACT + DVE lanes ----
        out_engs = [nc.sync, nc.gpsimd, nc.scalar]

        def affine_act(n, sl0=0, sl1=CH):
            gl = slice(n * CH + sl0, n * CH + sl1)
            nc.scalar.activation(
                yraw_sb[:, gl], accs[n][:, sl0:sl1], ACT.Relu,
                bias=shift_sb, scale=scale_sb,
            )
            return gl

        def affine_dve(n, sl0=0, sl1=CH):
            gl = slice(n * CH + sl0, n * CH + sl1)
            nc.vector.tensor_scalar(
                yraw_sb[:, gl], accs[n][:, sl0:sl1], scale_sb, shift_sb, ALU.mult,
                op1=ALU.add,
            )
            nc.vector.tensor_scalar_max(yraw_sb[:, gl], yraw_sb[:, gl], 0.0)
            return gl

        plan = [
            (affine_act, 0, 0, CH), (affine_dve, 1, 0, CH),
            (affine_act, 2, 0, CH), (affine_dve, 3, 0, CH),
            (affine_act, 4, 0, CH), (affine_dve, 5, 0, CH),
            (affine_act, 6, 0, CH // 2), (affine_dve, 6, CH // 2, CH),
        ]
        for i, (fn, n, a, b) in enumerate(plan):
            gl = fn(n, a, b)
            out_engs[i % 3].dma_start(out=yout[:, gl], in_=yraw_sb[:, gl])

    return nc


def build_nc():
    if "nc" not in _CACHE:
        nc = bacc.Bacc("TRN2", target_bir_lowering=False, debug=False)
        _emit(nc)
        nc.compile()
        _CACHE["nc"] = nc
    return _CACHE["nc"]


def make_in_maps(x, whole_w, rm_w, rm_q, bn_gamma, bn_beta):
    x = np.asarray(x, np.float32)
    whole_w = np.asarray(whole_w, np.float32)
    rm_w = np.asarray(rm_w, np.float32)
    rm_q = np.asarray(rm_q, np.float32)
    bn_gamma = np.asarray(bn_gamma, np.float32)
    bn_beta = np.asarray(bn_beta, np.float32)

    x0 = np.zeros((C, HP, HP), np.float32)
    x0[:, 1 : HP - 1, 1 : HP - 1] = x[0]
    x0 = x0.astype(np.float16)
    wc9 = whole_w.reshape(O, C, 9)
    rmt_a = rm_w.reshape(TH, 2, 128, 9).transpose(2, 1, 3, 0).reshape(128, KD * TH)
    rqt_a = rm_q.reshape(TH, C).T.reshape(2, 128, TH).transpose(1, 0, 2).reshape(128, 2 * TH)
    sigw_a = np.zeros((128, T), np.float32)
    for t in range(T):
        for h in range(HASH):
            sigw_a[t * HASH + h, t] = float(2 ** (HASH - 1 - h))
    blob16_a = np.ascontiguousarray(
        np.concatenate([rmt_a, rqt_a, sigw_a], axis=1)
    ).astype(np.float16)

    in_maps = []
    for core in range(N_CORES):
        o0 = core * OC
        wconv_a = np.ascontiguousarray(
            wc9[o0 : o0 + OC].reshape(OC, 2, 128, 9).transpose(2, 1, 3, 0)
        ).astype(np.float16)
        blob32_a = np.zeros((128, 2), np.float32)
        blob32_a[0:OC, 0] = bn_gamma[o0 : o0 + OC]
        blob32_a[0:OC, 1] = bn_beta[o0 : o0 + OC]
        in_maps.append(
            {
                "xin": x0,
                "wconv": wconv_a,
                "blob16": blob16_a,
                "blob32": blob32_a,
            }
        )
    return in_maps


def kernel(x, whole_w, rm_w, rm_q, bn_gamma, bn_beta):
    nc = build_nc()
    in_maps = make_in_maps(x, whole_w, rm_w, rm_q, bn_gamma, bn_beta)
    res = run_bass_kernel_spmd(nc, in_maps, list(range(N_CORES)))
    y = np.concatenate([r["yout"] for r in res.results], axis=0)
    return y.reshape(1, O, H, W).astype(np.float32)


# revision 12
# speedup vs baseline: 1.0911x; 1.0164x over previous
"""DynamicConv2d (moe_routing) TRN2 kernel: 64 out-channels/core x 8 cores, host gather.

Baseline structure + critical-path cuts: fp16 query-hash projection (2 PE
passes instead of 4 fp32), routing mask computed straight from PSUM, chunk-6
BN stats accumulated as two halves so the final stats op is half length, and
the sigq PSUM evacuation moved to the scalar engine off the DVE chain, and
input DMAs split across the sync/gpsimd rings (x ahead of weights) so the
first conv chunk's operands land earlier."""

import numpy as np
from contextlib import ExitStack

import concourse.bass as bass
import concourse.mybir as mybir
import concourse.tile as tile
from concourse import bacc
from concourse.bass_utils import run_bass_kernel_spmd

F32 = mybir.dt.float32
F16 = mybir.dt.float16
ALU = mybir.AluOpType
ACT = mybir.ActivationFunctionType

N_CORES = 8
O, C, KK, H, W = 512, 256, 3, 56, 56
OC = O // N_CORES
S = H * W
HP = H + 2
T, HASH = 10, 8
TH = T * HASH
D = C * KK * KK
KD = D // 128
NCH = 7
CH = S // NCH
SIZE_LIMIT = O // 2
EPS = 1e-3

_CACHE = {}


def _emit(nc):
    xin = nc.dram_tensor("xin", [C, HP, HP], F16, kind="ExternalInput").ap()
    wconv = nc.dram_tensor("wconv", [128, 2, 9, OC], F16, kind="ExternalInput").ap()
    rmt = nc.dram_tensor("rmt", [128, KD, TH], F16, kind="ExternalInput").ap()
    rqt = nc.dram_tensor("rqt", [128, 2, TH], F16, kind="ExternalInput").ap()
    sigw = nc.dram_tensor("sigw", [TH, T], F16, kind="ExternalInput").ap()
    gamma = nc.dram_tensor("gamma", [OC, 1], F32, kind="ExternalInput").ap()
    beta = nc.dram_tensor("beta", [OC, 1], F32, kind="ExternalInput").ap()
    yout = nc.dram_tensor("yout", [OC, S], F16, kind="ExternalOutput").ap()

    with tile.TileContext(nc) as tc, ExitStack() as ctx:
        consts = ctx.enter_context(tc.tile_pool(name="consts", bufs=1))
        work = ctx.enter_context(tc.tile_pool(name="work", bufs=1))
        scr = ctx.enter_context(tc.tile_pool(name="scr", bufs=2))
        pconv = ctx.enter_context(tc.tile_pool(name="pconv", bufs=7, space="PSUM"))
        psm = ctx.enter_context(tc.tile_pool(name="psm", bufs=1, space="PSUM"))

        wconv_sb = consts.tile([128, 2, 9, OC], F16)
        xpad = []
        for kc in range(2):
            xp = consts.tile([128, HP, HP], F16, tag=f"xpad{kc}", name=f"xp{kc}")
            xpad.append(xp)
        rings = [nc.sync, nc.gpsimd]
        # x first (the first conv matmul waits on it), then weights, per ring:
        # sync carries the low channel half, gpsimd the high half + tables.
        for kc in range(2):
            rings[kc].dma_start(
                out=xpad[kc][:, 0:10], in_=xin[kc * 128 : (kc + 1) * 128, 0:10]
            )
            rings[kc].dma_start(out=wconv_sb[:, kc], in_=wconv[:, kc])
        row_blocks = [(10, 18), (18, 26), (26, 34), (34, 42), (42, 50), (50, 58)]
        for r0, r1 in row_blocks:
            for kc in range(2):
                rings[kc].dma_start(
                    out=xpad[kc][:, r0:r1], in_=xin[kc * 128 : (kc + 1) * 128, r0:r1]
                )

        rmt_sb = consts.tile([128, KD, TH], F16)
        nc.gpsimd.dma_start(out=rmt_sb, in_=rmt)
        rqt_sb = consts.tile([128, 2, TH], F16)
        nc.gpsimd.dma_start(out=rqt_sb, in_=rqt)
        sigw_sb = consts.tile([TH, T], F16)
        nc.gpsimd.dma_start(out=sigw_sb, in_=sigw)
        gamma_sb = consts.tile([OC, 1], F32)
        nc.gpsimd.dma_start(out=gamma_sb, in_=gamma)
        beta_sb = consts.tile([OC, 1], F32)
        nc.gpsimd.dma_start(out=beta_sb, in_=beta)

        eps_sb = consts.tile([OC, 1], F32)
        nc.vector.memset(eps_sb, EPS)
        ones10_sb = consts.tile([T, 1], F16)
        nc.vector.memset(ones10_sb, 1.0)
        onesbc_sb = consts.tile([T, OC], F16)
        nc.vector.memset(onesbc_sb, 1.0)
        wu_l_sb = consts.tile([128, OC], F16)
        nc.vector.memset(wu_l_sb, 0.0)
        wu_r_sb = consts.tile([128, 448], F16)
        nc.vector.memset(wu_r_sb, 0.0)

        wu_ps = psm.tile([OC, 448], F32, tag="sp", name="wu")
        NWU = 8
        for i in range(NWU):
            nc.tensor.matmul(
                wu_ps, lhsT=wu_l_sb, rhs=wu_r_sb, start=(i == 0), stop=(i == NWU - 1)
            )

        yraw_sb = work.tile([OC, S], F16)
        stats_sb = work.tile([OC, 8, 6], F32)

        accs = {}

        def conv_chunk(n, splits=1):
            acc = pconv.tile([OC, CH], F32, tag="acc", name=f"acc{n}")
            accs[n] = acc
            i0 = 8 * n
            cw = CH // splits
            rw = 8 // splits
            for s_ in range(splits):
                for kc in range(2):
                    for t in range(9):
                        ky, kx = t // 3, t % 3
                        nc.tensor.matmul(
                            acc[:, s_ * cw : (s_ + 1) * cw],
                            lhsT=wconv_sb[:, kc, t, :],
                            rhs=xpad[kc][
                                :,
                                ky + i0 + rw * s_ : ky + i0 + rw * (s_ + 1),
                                kx : kx + W,
                            ],
                            start=(kc == 0 and t == 0),
                            stop=(kc == 1 and t == 8),
                        )
                nc.vector.bn_stats(
                    out=stats_sb[:, n + s_, :], in_=acc[:, s_ * cw : (s_ + 1) * cw]
                )
            if n != NCH - 1:
                nc.vector.tensor_copy(yraw_sb[:, n * CH : (n + 1) * CH], acc)

        for n in range(3):
            conv_chunk(n)

        qsumf_sb = work.tile([128, 2], F32)
        qsum_sb = work.tile([128, 2], F16)
        nc.vector.tensor_reduce(
            out=qsumf_sb[:, 0:1], in_=xpad[0], axis=mybir.AxisListType.XY, op=ALU.add
        )
        nc.vector.tensor_reduce(
            out=qsumf_sb[:, 1:2], in_=xpad[1], axis=mybir.AxisListType.XY, op=ALU.add
        )
        nc.vector.tensor_copy(qsum_sb, qsumf_sb)

        conv_chunk(3)
        conv_chunk(4)

        conv_chunk(5)
        conv_chunk(6, splits=2)

        projw_ps = psm.tile([TH, OC], F32, tag="sp", name="projw")
        for kc in range(2):
            for t in range(9):
                nc.tensor.matmul(
                    projw_ps,
                    lhsT=rmt_sb[:, kc * 9 + t, :],
                    rhs=wconv_sb[:, kc, t, :],
                    start=(kc == 0 and t == 0),
                    stop=(kc == 1 and t == 8),
                )

        bits_w = work.tile([TH, OC], F16)
        nc.vector.tensor_scalar(bits_w, projw_ps, 0.0, None, ALU.is_gt)

        sigw_ps = psm.tile([T, OC], F32, tag="sp", name="sigw")
        nc.tensor.matmul(sigw_ps, lhsT=sigw_sb, rhs=bits_w, start=True, stop=True)
        sigw_cp = work.tile([T, OC], F32)
        nc.vector.tensor_copy(sigw_cp, sigw_ps)

        projq_ps = psm.tile([TH, 1], F32, tag="sp", name="projq")
        for kc in range(2):
            nc.tensor.matmul(
                projq_ps,
                lhsT=rqt_sb[:, kc, :],
                rhs=qsum_sb[:, kc : kc + 1],
                start=(kc == 0),
                stop=(kc == 1),
            )
        bits_q = work.tile([TH, 1], F16)
        nc.vector.tensor_scalar(bits_q, projq_ps, 0.0, None, ALU.is_gt)
        sigq_ps = psm.tile([T, 1], F32, tag="sp", name="sigq")
        nc.tensor.matmul(sigq_ps, lhsT=sigw_sb, rhs=bits_q, start=True, stop=True)
        sigq_sb = work.tile([T, 1], F32)
        nc.scalar.copy(sigq_sb, sigq_ps)

        match_sb = work.tile([T, OC], F16)
        nc.vector.tensor_scalar(match_sb, sigw_cp, sigq_sb, None, ALU.is_equal)

        histc_ps = psm.tile([OC, 1], F32, tag="sp", name="histc")
        nc.tensor.matmul(
            histc_ps, lhsT=match_sb, rhs=ones10_sb, start=True, stop=True
        )
        mask_sb = work.tile([OC, 1], F32)
        nc.vector.tensor_scalar(mask_sb, histc_ps, 0.0, None, ALU.is_gt)

        mv_sb = work.tile([OC, 2], F32)
        nc.vector.bn_aggr(out=mv_sb, in_=stats_sb.rearrange("p a b -> p (a b)"))
        std_sb = work.tile([OC, 1], F32)
        nc.scalar.activation(std_sb, mv_sb[:, 1:2], ACT.Sqrt, bias=eps_sb)
        rstd_sb = work.tile([OC, 1], F32)
        nc.vector.reciprocal(rstd_sb, std_sb)
        scale_sb = work.tile([OC, 1], F32)
        nc.vector.scalar_tensor_tensor(
            out=scale_sb,
            in0=gamma_sb,
            scalar=rstd_sb,
            in1=mask_sb,
            op0=ALU.mult,
            op1=ALU.mult,
        )
        msc_sb = work.tile([OC, 1], F32)
        nc.vector.tensor_tensor(msc_sb, mv_sb[:, 0:1], scale_sb, ALU.mult)
        shift_sb = work.tile([OC, 1], F32)
        nc.vector.tensor_tensor(shift_sb, beta_sb, msc_sb, ALU.subtract)

        out_engs = [nc.sync, nc.gpsimd]

        def affine_act(n):
            sl = slice(n * CH, (n + 1) * CH)
            src_ap = accs[n] if n == NCH - 1 else yraw_sb[:, sl]
            nc.scalar.activation(
                yraw_sb[:, sl], src_ap, ACT.Relu, bias=shift_sb, scale=scale_sb
            )

        def affine_dve(n):
            sl = slice(n * CH, (n + 1) * CH)
            nc.vector.tensor_scalar(
                yraw_sb[:, sl], yraw_sb[:, sl], scale_sb, shift_sb, ALU.mult,
                op1=ALU.add,
            )
            nc.vector.tensor_scalar_max(yraw_sb[:, sl], yraw_sb[:, sl], 0.0)

        plan = [
            (6, affine_act), (5, affine_dve), (4, affine_dve),
            (3, affine_act), (2, affine_dve), (1, affine_dve),
            (0, affine_act),
        ]
        for i, (n, fn) in enumerate(plan):
            fn(n)
            sl = slice(n * CH, (n + 1) * CH)
            out_engs[i % 2].dma_start(out=yout[:, sl], in_=yraw_sb[:, sl])

    return nc


def build_nc():
    if "nc" not in _CACHE:
        nc = bacc.Bacc("TRN2", target_bir_lowering=False, debug=False)
        _emit(nc)
        nc.compile()
        _CACHE["nc"] = nc
    return _CACHE["nc"]


def make_in_maps(x, whole_w, rm_w, rm_q, bn_gamma, bn_beta):
    x = np.asarray(x, np.float32)
    whole_w = np.asarray(whole_w, np.float32)
    rm_w = np.asarray(rm_w, np.float32)
    rm_q = np.asarray(rm_q, np.float32)
    bn_gamma = np.asarray(bn_gamma, np.float32)
    bn_beta = np.asarray(bn_beta, np.float32)

    x0 = np.zeros((C, HP, HP), np.float32)
    x0[:, 1 : HP - 1, 1 : HP - 1] = x[0]
    x0 = x0.astype(np.float16)
    wc9 = whole_w.reshape(O, C, 9)
    rmt_a = np.ascontiguousarray(
        rm_w.reshape(TH, 2, 128, 9).transpose(2, 1, 3, 0).reshape(128, KD, TH)
    ).astype(np.float16)
    rqt_a = np.ascontiguousarray(
        rm_q.reshape(TH, C).T.reshape(2, 128, TH).transpose(1, 0, 2)
    ).astype(np.float16)
    sigw_a = np.zeros((TH, T), np.float32)
    for t in range(T):
        for h in range(HASH):
            sigw_a[t * HASH + h, t] = float(2 ** (HASH - 1 - h))
    sigw_a = sigw_a.astype(np.float16)

    in_maps = []
    for core in range(N_CORES):
        o0 = core * OC
        wconv_a = np.ascontiguousarray(
            wc9[o0 : o0 + OC].reshape(OC, 2, 128, 9).transpose(2, 1, 3, 0)
        ).astype(np.float16)
        in_maps.append(
            {
                "xin": x0,
                "wconv": wconv_a,
                "rmt": rmt_a,
                "rqt": rqt_a,
                "sigw": sigw_a,
                "gamma": np.ascontiguousarray(bn_gamma[o0 : o0 + OC, None]),
                "beta": np.ascontiguousarray(bn_beta[o0 : o0 + OC, None]),
            }
        )
    return in_maps


def kernel(x, whole_w, rm_w, rm_q, bn_gamma, bn_beta):
    nc = build_nc()
    in_maps = make_in_maps(x, whole_w, rm_w, rm_q, bn_gamma, bn_beta)
    res = run_bass_kernel_spmd(nc, in_maps, list(range(N_CORES)))
    y = np.concatenate([r["yout"] for r in res.results], axis=0)
    return y.reshape(1, O, H, W).astype(np.float32)


# revision 14
# speedup vs baseline: 1.1995x; 1.0993x over previous
"""DynamicConv2d (moe_routing) TRN2 kernel: 64 out-channels/core x 8 cores, host gather.

Baseline structure + critical-path cuts: fp16 query-hash projection (2 PE
passes instead of 4 fp32), routing mask computed straight from PSUM, chunk-6
BN stats accumulated as two halves so the final stats op is half length, and
the sigq PSUM evacuation moved to the scalar engine off the DVE chain."""

import numpy as np
from contextlib import ExitStack

import concourse.bass as bass
import concourse.mybir as mybir
import concourse.tile as tile
from concourse import bacc
from concourse.bass_utils import run_bass_kernel_spmd

F32 = mybir.dt.float32
F16 = mybir.dt.float16
ALU = mybir.AluOpType
ACT = mybir.ActivationFunctionType

N_CORES = 8
O, C, KK, H, W = 512, 256, 3, 56, 56
OC = O // N_CORES
S = H * W
HP = H + 2
T, HASH = 10, 8
TH = T * HASH
D = C * KK * KK
KD = D // 128
NCH = 7
CH = S // NCH
SIZE_LIMIT = O // 2
EPS = 1e-3

_CACHE = {}


def _emit(nc):
    xin = nc.dram_tensor("xin", [C, HP, HP], F16, kind="ExternalInput").ap()
    wconv = nc.dram_tensor("wconv", [128, 2, 9, OC], F16, kind="ExternalInput").ap()
    rmt = nc.dram_tensor("rmt", [128, KD, TH], F16, kind="ExternalInput").ap()
    rqt = nc.dram_tensor("rqt", [128, 2, TH], F16, kind="ExternalInput").ap()
    sigw = nc.dram_tensor("sigw", [TH, T], F16, kind="ExternalInput").ap()
    gamma = nc.dram_tensor("gamma", [OC, 1], F32, kind="ExternalInput").ap()
    beta = nc.dram_tensor("beta", [OC, 1], F32, kind="ExternalInput").ap()
    yout = nc.dram_tensor("yout", [OC, S], F16, kind="ExternalOutput").ap()

    with tile.TileContext(nc) as tc, ExitStack() as ctx:
        consts = ctx.enter_context(tc.tile_pool(name="consts", bufs=1))
        work = ctx.enter_context(tc.tile_pool(name="work", bufs=1))
        scr = ctx.enter_context(tc.tile_pool(name="scr", bufs=2))
        pconv = ctx.enter_context(tc.tile_pool(name="pconv", bufs=7, space="PSUM"))
        psm = ctx.enter_context(tc.tile_pool(name="psm", bufs=1, space="PSUM"))

        wconv_sb = consts.tile([128, 2, 9, OC], F16)
        xpad = []
        for kc in range(2):
            xp = consts.tile([128, HP, HP], F16, tag=f"xpad{kc}", name=f"xp{kc}")
            xpad.append(xp)
        for kc in range(2):
            nc.sync.dma_start(out=wconv_sb[:, kc], in_=wconv[:, kc])
            nc.sync.dma_start(
                out=xpad[kc][:, 0:10], in_=xin[kc * 128 : (kc + 1) * 128, 0:10]
            )
        row_blocks = [(10, 18), (18, 26), (26, 34), (34, 42), (42, 50), (50, 58)]
        for r0, r1 in row_blocks:
            for kc in range(2):
                nc.sync.dma_start(
                    out=xpad[kc][:, r0:r1], in_=xin[kc * 128 : (kc + 1) * 128, r0:r1]
                )

        rmt_sb = consts.tile([128, KD, TH], F16)
        nc.gpsimd.dma_start(out=rmt_sb, in_=rmt)
        rqt_sb = consts.tile([128, 2, TH], F16)
        nc.gpsimd.dma_start(out=rqt_sb, in_=rqt)
        sigw_sb = consts.tile([TH, T], F16)
        nc.gpsimd.dma_start(out=sigw_sb, in_=sigw)
        gamma_sb = consts.tile([OC, 1], F32)
        nc.gpsimd.dma_start(out=gamma_sb, in_=gamma)
        beta_sb = consts.tile([OC, 1], F32)
        nc.gpsimd.dma_start(out=beta_sb, in_=beta)

        eps_sb = consts.tile([OC, 1], F32)
        nc.vector.memset(eps_sb, EPS)
        ones10_sb = consts.tile([T, 1], F16)
        nc.vector.memset(ones10_sb, 1.0)
        onesbc_sb = consts.tile([T, OC], F16)
        nc.vector.memset(onesbc_sb, 1.0)
        wu_l_sb = consts.tile([128, OC], F16)
        nc.vector.memset(wu_l_sb, 0.0)
        wu_r_sb = consts.tile([128, 448], F16)
        nc.vector.memset(wu_r_sb, 0.0)

        wu_ps = psm.tile([OC, 448], F32, tag="sp", name="wu")
        NWU = 8
        for i in range(NWU):
            nc.tensor.matmul(
                wu_ps, lhsT=wu_l_sb, rhs=wu_r_sb, start=(i == 0), stop=(i == NWU - 1)
            )

        yraw_sb = work.tile([OC, S], F16)
        stats_sb = work.tile([OC, 8, 6], F32)

        accs = {}

        def conv_chunk(n, splits=1):
            acc = pconv.tile([OC, CH], F32, tag="acc", name=f"acc{n}")
            accs[n] = acc
            i0 = 8 * n
            cw = CH // splits
            rw = 8 // splits
            for s_ in range(splits):
                for kc in range(2):
                    for t in range(9):
                        ky, kx = t // 3, t % 3
                        nc.tensor.matmul(
                            acc[:, s_ * cw : (s_ + 1) * cw],
                            lhsT=wconv_sb[:, kc, t, :],
                            rhs=xpad[kc][
                                :,
                                ky + i0 + rw * s_ : ky + i0 + rw * (s_ + 1),
                                kx : kx + W,
                            ],
                            start=(kc == 0 and t == 0),
                            stop=(kc == 1 and t == 8),
                        )
                nc.vector.bn_stats(
                    out=stats_sb[:, n + s_, :], in_=acc[:, s_ * cw : (s_ + 1) * cw]
                )
            if n != NCH - 1:
                nc.vector.tensor_copy(yraw_sb[:, n * CH : (n + 1) * CH], acc)

        for n in range(3):
            conv_chunk(n)

        qsumf_sb = work.tile([128, 2], F32)
        qsum_sb = work.tile([128, 2], F16)
        nc.vector.tensor_reduce(
            out=qsumf_sb[:, 0:1], in_=xpad[0], axis=mybir.AxisListType.XY, op=ALU.add
        )
        nc.vector.tensor_reduce(
            out=qsumf_sb[:, 1:2], in_=xpad[1], axis=mybir.AxisListType.XY, op=ALU.add
        )
        nc.vector.tensor_copy(qsum_sb, qsumf_sb)

        conv_chunk(3)
        conv_chunk(4)

        projw_ps = psm.tile([TH, OC], F32, tag="sp", name="projw")
        for kc in range(2):
            for t in range(9):
                nc.tensor.matmul(
                    projw_ps,
                    lhsT=rmt_sb[:, kc * 9 + t, :],
                    rhs=wconv_sb[:, kc, t, :],
                    start=(kc == 0 and t == 0),
                    stop=(kc == 1 and t == 8),
                )
        bits_w = work.tile([TH, OC], F16)
        nc.vector.tensor_scalar(bits_w, projw_ps, 0.0, None, ALU.is_gt)

        conv_chunk(5)

        sigw_ps = psm.tile([T, OC], F32, tag="sp", name="sigw")
        nc.tensor.matmul(sigw_ps, lhsT=sigw_sb, rhs=bits_w, start=True, stop=True)
        sigw_cp = work.tile([T, OC], F32)
        nc.vector.tensor_copy(sigw_cp, sigw_ps)

        conv_chunk(6, splits=2)

        projq_ps = psm.tile([TH, 1], F32, tag="sp", name="projq")
        for kc in range(2):
            nc.tensor.matmul(
                projq_ps,
                lhsT=rqt_sb[:, kc, :],
                rhs=qsum_sb[:, kc : kc + 1],
                start=(kc == 0),
                stop=(kc == 1),
            )
        bits_q = work.tile([TH, 1], F16)
        nc.vector.tensor_scalar(bits_q, projq_ps, 0.0, None, ALU.is_gt)
        sigq_ps = psm.tile([T, 1], F32, tag="sp", name="sigq")
        nc.tensor.matmul(sigq_ps, lhsT=sigw_sb, rhs=bits_q, start=True, stop=True)
        sigq_sb = work.tile([T, 1], F32)
        nc.scalar.copy(sigq_sb, sigq_ps)

        match_sb = work.tile([T, OC], F16)
        nc.vector.tensor_scalar(match_sb, sigw_cp, sigq_sb, None, ALU.is_equal)

        histc_ps = psm.tile([OC, 1], F32, tag="sp", name="histc")
        nc.tensor.matmul(
            histc_ps, lhsT=match_sb, rhs=ones10_sb, start=True, stop=True
        )
        mask_sb = work.tile([OC, 1], F32)
        nc.vector.tensor_scalar(mask_sb, histc_ps, 0.0, None, ALU.is_gt)

        mv_sb = work.tile([OC, 2], F32)
        nc.vector.bn_aggr(out=mv_sb, in_=stats_sb.rearrange("p a b -> p (a b)"))
        std_sb = work.tile([OC, 1], F32)
        nc.scalar.activation(std_sb, mv_sb[:, 1:2], ACT.Sqrt, bias=eps_sb)
        rstd_sb = work.tile([OC, 1], F32)
        nc.vector.reciprocal(rstd_sb, std_sb)
        scale_sb = work.tile([OC, 1], F32)
        nc.vector.scalar_tensor_tensor(
            out=scale_sb,
            in0=gamma_sb,
            scalar=rstd_sb,
            in1=mask_sb,
            op0=ALU.mult,
            op1=ALU.mult,
        )
        msc_sb = work.tile([OC, 1], F32)
        nc.vector.tensor_tensor(msc_sb, mv_sb[:, 0:1], scale_sb, ALU.mult)
        shift_sb = work.tile([OC, 1], F32)
        nc.vector.tensor_tensor(shift_sb, beta_sb, msc_sb, ALU.subtract)

        out_engs = [nc.sync, nc.gpsimd]

        def affine_act(n):
            sl = slice(n * CH, (n + 1) * CH)
            src_ap = accs[n] if n == NCH - 1 else yraw_sb[:, sl]
            nc.scalar.activation(
                yraw_sb[:, sl], src_ap, ACT.Relu, bias=shift_sb, scale=scale_sb
            )

        def affine_dve(n):
            sl = slice(n * CH, (n + 1) * CH)
            nc.vector.tensor_scalar(
                yraw_sb[:, sl], yraw_sb[:, sl], scale_sb, shift_sb, ALU.mult,
                op1=ALU.add,
            )
            nc.vector.tensor_scalar_max(yraw_sb[:, sl], yraw_sb[:, sl], 0.0)

        plan = [
            (6, affine_act), (5, affine_dve), (4, affine_dve),
            (3, affine_act), (2, affine_dve), (1, affine_dve),
            (0, affine_act),
        ]
        for i, (n, fn) in enumerate(plan):
            fn(n)
            sl = slice(n * CH, (n + 1) * CH)
            out_engs[i % 2].dma_start(out=yout[:, sl], in_=yraw_sb[:, sl])

    return nc


def build_nc():
    if "nc" not in _CACHE:
        nc = bacc.Bacc("TRN2", target_bir_lowering=False, debug=False)
        _emit(nc)
        nc.compile()
        _CACHE["nc"] = nc
    return _CACHE["nc"]


def make_in_maps(x, whole_w, rm_w, rm_q, bn_gamma, bn_beta):
    x = np.asarray(x, np.float32)
    whole_w = np.asarray(whole_w, np.float32)
    rm_w = np.asarray(rm_w, np.float32)
    rm_q = np.asarray(rm_q, np.float32)
    bn_gamma = np.asarray(bn_gamma, np.float32)
    bn_beta = np.asarray(bn_beta, np.float32)

    x0 = np.zeros((C, HP, HP), np.float32)
    x0[:, 1 : HP - 1, 1 : HP - 1] = x[0]
    x0 = x0.astype(np.float16)
    wc9 = whole_w.reshape(O, C, 9)
    rmt_a = np.ascontiguousarray(
        rm_w.reshape(TH, 2, 128, 9).transpose(2, 1, 3, 0).reshape(128, KD, TH)
    ).astype(np.float16)
    rqt_a = np.ascontiguousarray(
        rm_q.reshape(TH, C).T.reshape(2, 128, TH).transpose(1, 0, 2)
    ).astype(np.float16)
    sigw_a = np.zeros((TH, T), np.float32)
    for t in range(T):
        for h in range(HASH):
            sigw_a[t * HASH + h, t] = float(2 ** (HASH - 1 - h))
    sigw_a = sigw_a.astype(np.float16)

    in_maps = []
    for core in range(N_CORES):
        o0 = core * OC
        wconv_a = np.ascontiguousarray(
            wc9[o0 : o0 + OC].reshape(OC, 2, 128, 9).transpose(2, 1, 3, 0)
        ).astype(np.float16)
        in_maps.append(
            {
                "xin": x0,
                "wconv": wconv_a,
                "rmt": rmt_a,
                "rqt": rqt_a,
                "sigw": sigw_a,
                "gamma": np.ascontiguousarray(bn_gamma[o0 : o0 + OC, None]),
                "beta": np.ascontiguousarray(bn_beta[o0 : o0 + OC, None]),
            }
        )
    return in_maps


def kernel(x, whole_w, rm_w, rm_q, bn_gamma, bn_beta):
    nc = build_nc()
    in_maps = make_in_maps(x, whole_w, rm_w, rm_q, bn_gamma, bn_beta)
    res = run_bass_kernel_spmd(nc, in_maps, list(range(N_CORES)))
    y = np.concatenate([r["yout"] for r in res.results], axis=0)
    return y.reshape(1, O, H, W).astype(np.float32)
